# revision 8
# baseline (speedup 1.0000x reference)
"""Trainium2 Bass kernel for nn_ClassicalHybridClassifier.

Pipeline: conv1(5x5,s2) -> maxpool(2,s1) -> conv2(3x3,s2) -> maxpool(2,s1)
          -> fc1 [120,55815] -> fc2 -> fc3 -> qnn tanh stack -> RBF vs 8192
          train states -> [1,2] output.

Sharding: each of the 8 cores computes a horizontal band of the conv pipeline
(bands over the 61 pool2 output rows: 8,8,8,8,8,7,7,7) and the matching
contraction slice of fc1 (tensor-parallel over fc1's 55815 input dim, weights
restructured host-side to match the on-chip feature layout). One AllReduce of
the [120,10] fc1 partials; the tiny tail (fc2/fc3/qnn/RBF over all 8192 train
states) is replicated on every core.

Convs are expressed as banded-weight matmuls: contraction over (channel,
input row) with the 5 (resp. 3) kernel-column taps accumulated in PSUM via
column-shifted strided views of the input rows. Vertical max-pools cross the
partition dim, handled by a partition-shift matmul. fc1 runs as 61 per-column
matmul triples in split-bf16 (hi/lo) for ~fp32 accuracy at bf16 speed.
"""

import numpy as np
import ml_dtypes

import concourse.bass as bass
import concourse.mybir as mybir
import concourse.tile as tile
from concourse import bass_utils, bacc

F32 = mybir.dt.float32
F32R = mybir.dt.float32r
BF16 = mybir.dt.bfloat16
AF = mybir.ActivationFunctionType
ALU = mybir.AluOpType
AX = mybir.AxisListType

N_CORES = 8
BANDS = [(0, 8), (8, 16), (16, 24), (24, 32), (32, 40), (40, 47), (47, 54), (54, 61)]

B = 10          # batch
XR = 43         # x rows per core (padded)
XC = 252        # x cols incl 1+1 zero pad
C1R = 20        # conv1 out rows per core (padded)
P1R = 19        # pool1 rows per core (padded)
C2R = 9         # conv2 out rows per core (padded)
NJ = 61         # pool2 / fc1 spatial columns
C1CH = 6
C2CH = 15

# conv1 N chunking over images (PSUM bank = 512 fp32)
C1_CHUNKS = [(0, 4), (4, 3), (7, 3)]     # (img0, nimg): 4*124=496, 3*124=372
C2_CHUNKS = [(0, 8), (8, 2)]             # 8*62=496, 2*62=124
P2_CHUNKS = [(0, 8), (8, 2)]             # over (img, 61): 488, 122
SH_CHUNKS = [(0, 4), (4, 4), (8, 2)]     # shift-mm chunks: even N (492, 492, 246)


def _build_nc():
    nc = bacc.Bacc("TRN2", target_bir_lowering=False, debug=False,
                   num_devices=N_CORES)

    d = {}
    def din(name, shape, dt):
        d[name] = nc.dram_tensor(name, list(shape), dt, kind="ExternalInput").ap()

    din("x_slab", (3, XR, B, XC), F32R)
    din("w1a", (87, 5, 120), F32R)
    din("w1b", (43, 5, 120), F32R)
    din("w2", (115, 3, 135), F32R)
    din("s1m", (120, 114), F32R)
    din("s2a", (120, 120), F32R)
    din("s2b", (15, 120), F32R)
    din("wslab", (120, NJ, 2, 120), BF16)
    din("fc1b", (120, 1), F32)
    din("w2fcT", (120, 84), F32)
    din("fc2b", (84, 1), F32)
    din("w3fcT", (84, 1), F32)
    din("b3vec", (B, 1), F32)
    din("wq1T", (B, 20), F32)
    din("wq2T", (20, 5), F32)
    din("idt10", (B, B), F32)
    din("ts_r", (128, 64, 5), F32)
    din("kcls_r", (128, 2, 64), F32)
    din("kclsb", (1, 2), F32)
    din("ones_bx", (1, B, XC), F32R)
    din("ones_v", (1, B, 125), F32R)
    din("zpad_v", (114, B, 2), F32R)

    out_d = nc.dram_tensor("out", [1, 2], F32, kind="ExternalOutput").ap()
    warm_d = nc.dram_tensor("warm", [1, 4], F32, kind="ExternalOutput").ap()

    with tile.TileContext(nc) as tc:
        with (
            tc.tile_pool(name="sb", bufs=1) as sb,
            tc.tile_pool(name="dr", bufs=1, space="DRAM") as dr,
        ):
            # ---- DMAs in ----
            wslab_t = sb.tile([120, NJ, 2, 120], BF16)
            nc.sync.dma_start(wslab_t[:], d["wslab"][:])

            x_a = sb.tile([87, B, XC], F32R)       # (c0,c1 rows) + ones row 86
            x_b = sb.tile([43, B, XC], F32R)       # c2 rows
            nc.sync.dma_start(x_a[0:86, :, :], d["x_slab"][0:2, :, :, :])
            nc.sync.dma_start(x_b[:], d["x_slab"][2, :, :, :])
            nc.sync.dma_start(x_a[86:87, :, :], d["ones_bx"][:])

            w1a_t = sb.tile([87, 5, 120], F32R)
            w1b_t = sb.tile([43, 5, 120], F32R)
            w2_t = sb.tile([115, 3, 135], F32R)
            s1m_t = sb.tile([120, 114], F32R)
            s2a_t = sb.tile([120, 120], F32R)
            s2b_t = sb.tile([15, 120], F32R)
            for nm, t in (("w1a", w1a_t), ("w1b", w1b_t), ("w2", w2_t),
                          ("s1m", s1m_t), ("s2a", s2a_t), ("s2b", s2b_t)):
                nc.sync.dma_start(t[:], d[nm][:])

            small = {}
            for nm, shape in (("fc1b", (120, 1)), ("w2fcT", (120, 84)),
                              ("fc2b", (84, 1)), ("w3fcT", (84, 1)),
                              ("b3vec", (B, 1)), ("wq1T", (B, 20)),
                              ("wq2T", (20, 5)), ("idt10", (B, B)),
                              ("ts_r", (128, 64, 5)), ("kcls_r", (128, 2, 64)),
                              ("kclsb", (1, 2))):
                t = sb.tile(list(shape), F32, name=f"sb_{nm}")
                nc.sync.dma_start(t[:], d[nm][:])
                small[nm] = t

            # ---- PE warmup during input DMA ----
            with tc.tile_pool(name="ps_w", bufs=1, space="PSUM") as ps_w:
                wsc = sb.tile([128, 512], BF16)
                nc.vector.memset(wsc[:], 0.0)
                wps = ps_w.tile([128, 512], F32)
                for i in range(10):
                    nc.tensor.matmul(wps[:, 0:512], wsc[:, 0:128], wsc[:],
                                     start=(i == 0), stop=(i == 9))
                wout = sb.tile([1, 4], F32)
                nc.vector.tensor_copy(wout[:], wps[0:1, 0:4])
                nc.sync.dma_start(warm_d[:], wout[:])

            # ---- conv1 + pool1 ----
            Cs = sb.tile([120, B, 124], F32R)      # conv1 psum eviction
            Ch = sb.tile([120, B, 123], F32R)      # horizontal max
            V = sb.tile([128, B, 125], F32R)       # pool1 out, (py,ich) + ones@114

            with tc.tile_pool(name="ps_1", bufs=1, space="PSUM") as ps1:
                Cp = ps1.tile([120, 1536], F32)    # conv1 psum, 3 banks
                Sh = ps1.tile([114, 1536], F32)    # shifted Ch

                for ci, (i0, ni) in enumerate(C1_CHUNKS):
                    nmm = 10
                    k = 0
                    for kx in range(5):
                        for grp in range(2):
                            xt, wt, kdim = ((x_a, w1a_t, 87) if grp == 0
                                            else (x_b, w1b_t, 43))
                            rhs = xt[0:kdim, i0:i0 + ni, kx:kx + 248:2]
                            nc.tensor.matmul(
                                Cp[:, ci * 512: ci * 512 + ni * 124],
                                wt[0:kdim, kx, :], rhs,
                                start=(k == 0), stop=(k == nmm - 1))
                            k += 1
                    cv = Cp[:, ci * 512: ci * 512 + ni * 124].rearrange(
                        "p (i x) -> p i x", x=124)
                    # evict on ACT, then horizontal pool max (one PSUM operand)
                    nc.scalar.copy(Cs[:, i0:i0 + ni, :], cv)
                    nc.vector.tensor_max(Ch[:, i0:i0 + ni, :],
                                         Cs[:, i0:i0 + ni, 0:123],
                                         cv[:, :, 1:124])

                # vertical pool via partition-shift matmul; V = max(Ch,0,Sh)
                nc.sync.dma_start(V[114:115, :, :], d["ones_v"][:])
                nc.sync.dma_start(V[0:114, :, 0:1], d["zpad_v"][:, :, 0:1])
                nc.sync.dma_start(V[0:114, :, 124:125], d["zpad_v"][:, :, 1:2])
                for ci, (i0, ni) in enumerate(SH_CHUNKS):
                    nc.tensor.matmul(
                        Sh[:, ci * 512: ci * 512 + ni * 123],
                        s1m_t[:], Ch[:, i0:i0 + ni, :],
                        start=True, stop=True)
                    sv = Sh[:, ci * 512: ci * 512 + ni * 123].rearrange(
                        "p (i x) -> p i x", x=123)
                    nc.vector.scalar_tensor_tensor(
                        V[0:114, i0:i0 + ni, 1:124],
                        Ch[0:114, i0:i0 + ni, :], 0.0, sv[0:114, :, :],
                        op0=ALU.max, op1=ALU.max)

            # ---- conv2 + pool2 ----
            C2s_a = sb.tile([120, B, 62], F32R)
            C2s_b = sb.tile([15, B, 62], F32R)
            C2h_a = sb.tile([120, B, 61], F32R)
            C2h_b = sb.tile([15, B, 61], F32R)
            V2 = sb.tile([120, B, NJ], F32)

            with tc.tile_pool(name="ps_2", bufs=1, space="PSUM") as ps2:
                C2a = ps2.tile([120, 1024], F32)
                C2b = ps2.tile([15, 1024], F32)
                Sh2 = ps2.tile([120, 1024], F32)

                for ci, (i0, ni) in enumerate(C2_CHUNKS):
                    for grp, (cp, m0, m1) in enumerate(
                            ((C2a, 0, 120), (C2b, 120, 135))):
                        for kxp in range(3):
                            rhs = V[0:115, i0:i0 + ni, kxp:kxp + 123:2]
                            nc.tensor.matmul(
                                cp[:, ci * 512: ci * 512 + ni * 62],
                                w2_t[:, kxp, m0:m1], rhs,
                                start=(kxp == 0), stop=(kxp == 2))
                    for cp, cs, ch in ((C2a, C2s_a, C2h_a), (C2b, C2s_b, C2h_b)):
                        cv = cp[:, ci * 512: ci * 512 + ni * 62].rearrange(
                            "p (i x) -> p i x", x=62)
                        nc.scalar.copy(cs[:, i0:i0 + ni, :], cv)
                        # relu + horizontal pool (one PSUM operand)
                        nc.vector.scalar_tensor_tensor(
                            ch[:, i0:i0 + ni, :],
                            cs[:, i0:i0 + ni, 0:61], 0.0, cv[:, :, 1:62],
                            op0=ALU.max, op1=ALU.max)

                for ci, (i0, ni) in enumerate(P2_CHUNKS):
                    nc.tensor.matmul(
                        Sh2[:, ci * 512: ci * 512 + ni * 61],
                        s2a_t[:], C2h_a[:, i0:i0 + ni, :],
                        start=True, stop=False)
                    nc.tensor.matmul(
                        Sh2[:, ci * 512: ci * 512 + ni * 61],
                        s2b_t[:], C2h_b[:, i0:i0 + ni, :],
                        start=False, stop=True)
                    sv = Sh2[:, ci * 512: ci * 512 + ni * 61].rearrange(
                        "p (i x) -> p i x", x=61)
                    nc.vector.tensor_max(V2[:, i0:i0 + ni, :],
                                         C2h_a[:, i0:i0 + ni, :], sv)

            # ---- fc1 (split-bf16, tensor-parallel contraction) ----
            h_hi = sb.tile([120, NJ, B], BF16)
            h_lo = sb.tile([120, NJ, B], BF16)
            v2t = V2[:].rearrange("p i j -> p j i")
            nc.vector.tensor_copy(h_hi[:], v2t)
            nc.vector.tensor_sub(h_lo[:], v2t, h_hi[:])

            fc1s = sb.tile([B, 120], F32)
            with tc.tile_pool(name="ps_3", bufs=1, space="PSUM") as ps3:
                fps = ps3.tile([B, 120], F32)
                for j in range(NJ):
                    nc.tensor.matmul(fps[:], h_hi[:, j, :], wslab_t[:, j, 0, :],
                                     start=(j == 0), stop=False)
                    nc.tensor.matmul(fps[:], h_lo[:, j, :], wslab_t[:, j, 0, :],
                                     start=False, stop=False)
                    nc.tensor.matmul(fps[:], h_hi[:, j, :], wslab_t[:, j, 1, :],
                                     start=False, stop=(j == NJ - 1))
                nc.vector.tensor_copy(fc1s[:], fps[:])

            # ---- AllReduce fc1 partials ----
            arin = dr.tile([B, 120], F32)
            arout = dr.tile([B, 120], F32, addr_space="Shared")
            nc.sync.dma_start(arin[:], fc1s[:])
            nc.gpsimd.collective_compute(
                "AllReduce", ALU.add,
                replica_groups=[list(range(N_CORES))],
                ins=[arin.opt()], outs=[arout.opt()])
            h1post = sb.tile([B, 120], F32)
            nc.sync.dma_start(h1post[:], arout[:])

            # ---- tail (replicated) ----
            h1 = sb.tile([120, B], F32)
            h2 = sb.tile([84, B], F32)
            h10 = sb.tile([B, 1], F32)
            s1 = sb.tile([20, 1], F32)
            fs_row = sb.tile([1, 5], F32)
            fsb = sb.tile([128, 5], F32)
            diff = sb.tile([128, 64, 5], F32)
            sq = sb.tile([128, 64, 5], F32)
            d2 = sb.tile([128, 64], F32)
            kxv = sb.tile([128, 64], F32)
            pr = sb.tile([128, 2, 64], F32)
            krw = sb.tile([128, 2], F32)
            ones_t = sb.tile([128, 1], F32)
            out_sb = sb.tile([1, 2], F32)
            nc.vector.memset(ones_t[:], 1.0)

            with tc.tile_pool(name="ps_4", bufs=1, space="PSUM") as ps4:
                tp = ps4.tile([120, B], F32)
                nc.tensor.transpose(tp[:], h1post[:], small["idt10"][:])
                nc.scalar.activation(h1[:], tp[:], AF.Relu,
                                     bias=small["fc1b"][:])

                p2 = ps4.tile([84, B], F32)
                nc.tensor.matmul(p2[:], small["w2fcT"][:], h1[:],
                                 start=True, stop=True)
                nc.scalar.activation(h2[:], p2[:], AF.Relu,
                                     bias=small["fc2b"][:])

                p3 = ps4.tile([B, 1], F32)
                nc.tensor.matmul(p3[:], h2[:], small["w3fcT"][:],
                                 start=True, stop=True)
                nc.scalar.activation(h10[:], p3[:], AF.Identity,
                                     bias=small["b3vec"][:])

                p4 = ps4.tile([20, 1], F32)
                nc.tensor.matmul(p4[:], small["wq1T"][:], h10[:],
                                 start=True, stop=True)
                nc.scalar.activation(s1[:], p4[:], AF.Tanh)

                p5 = ps4.tile([1, 5], F32)
                nc.tensor.matmul(p5[:], s1[:], small["wq2T"][:],
                                 start=True, stop=True)
                nc.scalar.activation(fs_row[:], p5[:], AF.Tanh)

                nc.gpsimd.partition_broadcast(fsb[:], fs_row[0:1, :])
                nc.vector.tensor_sub(
                    diff[:], small["ts_r"][:],
                    fsb[:].unsqueeze(1).broadcast_to([128, 64, 5]))
                nc.scalar.square(sq[:], diff[:])
                nc.vector.reduce_sum(d2[:], sq[:], axis=AX.X)
                nc.scalar.activation(kxv[:], d2[:], AF.Exp, scale=-1.0)
                nc.vector.tensor_mul(
                    pr[:], small["kcls_r"][:],
                    kxv[:].unsqueeze(1).broadcast_to([128, 2, 64]))
                nc.vector.reduce_sum(krw[:], pr[:], axis=AX.X)

                p6 = ps4.tile([1, 2], F32)
                nc.tensor.matmul(p6[:], ones_t[:], krw[:],
                                 start=True, stop=True)
                nc.vector.tensor_add(out_sb[:], p6[:], small["kclsb"][:])

            nc.sync.dma_start(out_d[:], out_sb[:])

    nc.compile()
    return nc


def _prep_inputs(inputs):
    f32 = np.float32
    x = np.asarray(inputs["x"], f32)
    conv1_w = np.asarray(inputs["conv1_w"], f32)
    conv1_b = np.asarray(inputs["conv1_b"], f32)
    conv2_w = np.asarray(inputs["conv2_w"], f32)
    conv2_b = np.asarray(inputs["conv2_b"], f32)
    fc1_w = np.asarray(inputs["fc1_w"], f32)
    fc1_b = np.asarray(inputs["fc1_b"], f32)
    fc2_w = np.asarray(inputs["fc2_w"], f32)
    fc2_b = np.asarray(inputs["fc2_b"], f32)
    fc3_w = np.asarray(inputs["fc3_w"], f32)
    fc3_b = np.asarray(inputs["fc3_b"], f32)
    qnn_w1 = np.asarray(inputs["qnn_w1"], f32)
    qnn_w2 = np.asarray(inputs["qnn_w2"], f32)
    ts = np.asarray(inputs["train_states"], f32)
    kcls_w = np.asarray(inputs["kcls_w"], f32)
    kcls_b = np.asarray(inputs["kcls_b"], f32)

    shared = {
        "fc1b": fc1_b.reshape(120, 1),
        "w2fcT": np.ascontiguousarray(fc2_w.T),
        "fc2b": fc2_b.reshape(84, 1),
        "w3fcT": np.ascontiguousarray(fc3_w.T),
        "b3vec": np.full((B, 1), fc3_b[0], f32),
        "wq1T": np.ascontiguousarray(qnn_w1.T),
        "wq2T": np.ascontiguousarray(qnn_w2.T),
        "idt10": np.eye(B, dtype=f32),
        "ts_r": np.ascontiguousarray(ts.reshape(128, 64, 5)),
        "kcls_r": np.ascontiguousarray(
            kcls_w.reshape(2, 128, 64).transpose(1, 0, 2)),
        "kclsb": kcls_b.reshape(1, 2),
    }

    fc1_w4 = fc1_w.reshape(120, 15, 61, 61)

    in_maps = []
    for a, b in BANDS:
        nb = b - a
        Y0 = 2 * a - 1          # conv1 row of y_loc 0 (also pool1 row of py_loc 0)
        X0 = 4 * a - 3          # x row of r_loc 0

        # x slab [3, XR, B, XC], zero-padded rows/cols
        xs = np.zeros((3, XR, B, XC), f32)
        r_lo = max(0, X0)
        r_hi = min(250, X0 + XR)
        xs[:, r_lo - X0: r_hi - X0, :, 1:251] = (
            x[:, :, r_lo:r_hi, :].transpose(1, 2, 0, 3))

        # conv1 banded weights: K=(c, r_loc)+bias, M=(y_loc, och), per kx
        w1 = np.zeros((3, 43, 5, 120), f32)     # [c, r_loc, kx, m=(y_loc,och)]
        for y_loc in range(C1R):
            y = Y0 + y_loc
            if not (0 <= y <= 123):
                continue
            for ky in range(5):
                r_loc = 2 * y_loc + ky
                if r_loc >= XR:
                    continue
                for c in range(3):
                    w1[c, r_loc, :, y_loc * 6: y_loc * 6 + 6] = \
                        conv1_w[:, c, ky, :].T
        w1a = np.zeros((87, 5, 120), f32)
        w1a[0:43] = w1[0]
        w1a[43:86] = w1[1]
        w1a[86, 0, :] = np.tile(conv1_b, C1R)   # bias row, kx=0 only
        w1b = np.ascontiguousarray(w1[2])

        # conv2 banded weights: K=(py_loc, ich)+bias@114, M=(i2_loc, och2)
        w2 = np.zeros((115, 3, 135), f32)
        for i2_loc in range(C2R):
            i2 = a + i2_loc
            if i2 > 61:
                continue
            for kyp in range(3):
                py_loc = 2 * i2_loc + kyp
                py = Y0 + py_loc
                if py_loc >= P1R or not (0 <= py <= 122):
                    continue
                for ich in range(6):
                    q = py_loc * 6 + ich
                    m0 = i2_loc * 15
                    w2[q, :, m0:m0 + 15] = conv2_w[:, ich, kyp, :].T
        w2[114, 0, :] = np.tile(conv2_b, 9)     # bias row, kxp=0 only

        # partition-shift matrices
        s1m = np.zeros((120, 114), f32)
        for m in range(114):
            s1m[m + 6, m] = 1.0
        s2a = np.zeros((120, 120), f32)
        s2b = np.zeros((15, 120), f32)
        for m in range(105):
            s2a[m + 15, m] = 1.0
        for m in range(105, 120):
            s2b[m - 105, m] = 1.0

        # fc1 weight slab, split bf16: [p=(i2_loc,och2), j, {hi,lo}, och1]
        wsl = np.zeros((8, 15, NJ, 120), f32)
        nrow = min(nb, 8)
        wsl[0:nrow] = fc1_w4[:, :, a:a + nrow, :].transpose(2, 1, 3, 0)
        wsl = wsl.reshape(120, NJ, 120)
        hi = wsl.astype(ml_dtypes.bfloat16)
        lo = (wsl - hi.astype(f32)).astype(ml_dtypes.bfloat16)
        wslab = np.stack([hi, lo], axis=2)      # [120, NJ, 2, 120] bf16

        m = dict(shared)
        m["ones_bx"] = np.ones((1, B, XC), f32)
        m["ones_v"] = np.ones((1, B, 125), f32)
        m["zpad_v"] = np.zeros((114, B, 2), f32)
        m.update({"x_slab": xs, "w1a": w1a, "w1b": w1b, "w2": w2,
                  "s1m": s1m, "s2a": s2a, "s2b": s2b,
                  "wslab": np.ascontiguousarray(wslab)})
        in_maps.append(m)
    return in_maps


_NC_CACHE = None


def kernel(**inputs) -> np.ndarray:
    global _NC_CACHE
    if _NC_CACHE is None:
        _NC_CACHE = _build_nc()
    nc = _NC_CACHE
    in_maps = _prep_inputs(inputs)
    res = bass_utils.run_bass_kernel_spmd(
        nc, in_maps, core_ids=list(range(N_CORES)))
    return res.results[0]["out"]


# revision 9
# speedup vs baseline: 1.0999x; 1.0999x over previous
"""Trainium2 Bass kernel for nn_ClassicalHybridClassifier.

Pipeline: conv1(5x5,s2) -> maxpool(2,s1) -> conv2(3x3,s2) -> maxpool(2,s1)
          -> fc1 [120,55815] -> fc2 -> fc3 -> qnn tanh stack -> RBF vs 8192
          train states -> [1,2] output.

Sharding: each of the 8 cores computes a horizontal band of the conv pipeline
(bands over the 61 pool2 output rows: 8,8,8,8,8,7,7,7) and the matching
contraction slice of fc1 (tensor-parallel over fc1's 55815 input dim, weights
restructured host-side to match the on-chip feature layout). One AllReduce of
the [120,10] fc1 partials; the tiny tail (fc2/fc3/qnn/RBF over all 8192 train
states) is replicated on every core.

Convs are expressed as banded-weight matmuls: contraction over (channel,
input row) with the 5 (resp. 3) kernel-column taps accumulated in PSUM via
column-shifted strided views of the input rows. Vertical max-pools cross the
partition dim, handled by a partition-shift matmul. fc1 runs as 61 per-column
matmul triples in split-bf16 (hi/lo) for ~fp32 accuracy at bf16 speed.
"""

import numpy as np
import ml_dtypes

import concourse.bass as bass
import concourse.mybir as mybir
import concourse.tile as tile
from concourse import bass_utils, bacc

F32 = mybir.dt.float32
F32R = mybir.dt.float32r
BF16 = mybir.dt.bfloat16
AF = mybir.ActivationFunctionType
ALU = mybir.AluOpType
AX = mybir.AxisListType

N_CORES = 8
BANDS = [(0, 8), (8, 16), (16, 24), (24, 32), (32, 40), (40, 47), (47, 54), (54, 61)]

B = 10          # batch
XR = 43         # x rows per core (padded)
XC = 252        # x cols incl 1+1 zero pad
C1R = 20        # conv1 out rows per core (padded)
P1R = 19        # pool1 rows per core (padded)
C2R = 9         # conv2 out rows per core (padded)
NJ = 61         # pool2 / fc1 spatial columns
C1CH = 6
C2CH = 15

# conv1 N chunking over images (PSUM bank = 512 fp32)
C1_CHUNKS = [(0, 4), (4, 3), (7, 3)]     # (img0, nimg): 4*124=496, 3*124=372
C2_CHUNKS = [(0, 8), (8, 2)]             # 8*62=496, 2*62=124
P2_CHUNKS = [(0, 8), (8, 2)]             # over (img, 61): 488, 122
SH_CHUNKS = [(0, 4), (4, 4), (8, 2)]     # shift-mm chunks: even N (492, 492, 246)


def _build_nc():
    nc = bacc.Bacc("TRN2", target_bir_lowering=False, debug=False,
                   num_devices=N_CORES)

    d = {}
    def din(name, shape, dt):
        d[name] = nc.dram_tensor(name, list(shape), dt, kind="ExternalInput").ap()

    din("x_slab", (3, XR, B, XC), F32R)
    din("w1a", (87, 5, 120), F32R)
    din("w1b", (43, 5, 120), F32R)
    din("w2", (115, 3, 135), F32R)
    din("s1m", (120, 114), F32R)
    din("s2a", (120, 120), F32R)
    din("s2b", (15, 120), F32R)
    din("wslab", (120, NJ, 2, 120), BF16)
    din("fc1b", (120, 1), F32)
    din("w2fcT", (120, 84), F32)
    din("fc2b", (84, 1), F32)
    din("w3fcT", (84, 1), F32)
    din("b3vec", (B, 1), F32)
    din("wq1T", (B, 20), F32)
    din("wq2T", (20, 5), F32)
    din("idt10", (B, B), F32)
    din("ts_r", (128, 64, 5), F32)
    din("kcls_r", (128, 2, 64), F32)
    din("kclsb", (1, 2), F32)
    din("ones_bx", (1, B, XC), F32R)
    din("ones_v", (1, B, 125), F32R)
    din("zpad_v", (114, B, 2), F32R)

    out_d = nc.dram_tensor("out", [1, 2], F32, kind="ExternalOutput").ap()
    warm_d = nc.dram_tensor("warm", [1, 4], F32, kind="ExternalOutput").ap()

    with tile.TileContext(nc) as tc:
        with (
            tc.tile_pool(name="sb", bufs=1) as sb,
            tc.tile_pool(name="dr", bufs=1, space="DRAM") as dr,
        ):
            # ---- DMAs in ----
            wslab_t = sb.tile([120, NJ, 2, 120], BF16)
            nc.scalar.dma_start(wslab_t[:], d["wslab"][:])

            x_a = sb.tile([87, B, XC], F32R)       # (c0,c1 rows) + ones row 86
            x_b = sb.tile([43, B, XC], F32R)       # c2 rows
            nc.sync.dma_start(x_a[0:86, :, :], d["x_slab"][0:2, :, :, :])
            nc.sync.dma_start(x_b[:], d["x_slab"][2, :, :, :])
            nc.sync.dma_start(x_a[86:87, :, :], d["ones_bx"][:])

            w1a_t = sb.tile([87, 5, 120], F32R)
            w1b_t = sb.tile([43, 5, 120], F32R)
            w2_t = sb.tile([115, 3, 135], F32R)
            s1m_t = sb.tile([120, 114], F32R)
            s2a_t = sb.tile([120, 120], F32R)
            s2b_t = sb.tile([15, 120], F32R)
            for nm, t in (("w1a", w1a_t), ("w1b", w1b_t), ("s1m", s1m_t)):
                nc.sync.dma_start(t[:], d[nm][:])
            for nm, t in (("w2", w2_t), ("s2a", s2a_t), ("s2b", s2b_t)):
                nc.gpsimd.dma_start(t[:], d[nm][:])

            small = {}
            for nm, shape in (("fc1b", (120, 1)), ("w2fcT", (120, 84)),
                              ("fc2b", (84, 1)), ("w3fcT", (84, 1)),
                              ("b3vec", (B, 1)), ("wq1T", (B, 20)),
                              ("wq2T", (20, 5)), ("idt10", (B, B)),
                              ("ts_r", (128, 64, 5)), ("kcls_r", (128, 2, 64)),
                              ("kclsb", (1, 2))):
                t = sb.tile(list(shape), F32, name=f"sb_{nm}")
                nc.gpsimd.dma_start(t[:], d[nm][:])
                small[nm] = t

            # ---- PE warmup during input DMA ----
            with tc.tile_pool(name="ps_w", bufs=1, space="PSUM") as ps_w:
                wsc = sb.tile([128, 512], BF16)
                nc.vector.memset(wsc[:], 0.0)
                wps = ps_w.tile([128, 512], F32)
                for i in range(10):
                    nc.tensor.matmul(wps[:, 0:512], wsc[:, 0:128], wsc[:],
                                     start=(i == 0), stop=(i == 9))
                wout = sb.tile([1, 4], F32)
                nc.vector.tensor_copy(wout[:], wps[0:1, 0:4])
                nc.sync.dma_start(warm_d[:], wout[:])

            # ---- conv1 + pool1 ----
            Cs = sb.tile([120, B, 124], F32R)      # conv1 psum eviction
            Ch = sb.tile([120, B, 123], F32R)      # horizontal max
            V = sb.tile([128, B, 125], F32R)       # pool1 out, (py,ich) + ones@114

            with tc.tile_pool(name="ps_1", bufs=1, space="PSUM") as ps1:
                Cp = ps1.tile([120, 1536], F32)    # conv1 psum, 3 banks
                Sh = ps1.tile([114, 1536], F32)    # shifted Ch

                for ci, (i0, ni) in enumerate(C1_CHUNKS):
                    nmm = 10
                    k = 0
                    for kx in range(5):
                        for grp in range(2):
                            xt, wt, kdim = ((x_a, w1a_t, 87) if grp == 0
                                            else (x_b, w1b_t, 43))
                            rhs = xt[0:kdim, i0:i0 + ni, kx:kx + 248:2]
                            nc.tensor.matmul(
                                Cp[:, ci * 512: ci * 512 + ni * 124],
                                wt[0:kdim, kx, :], rhs,
                                start=(k == 0), stop=(k == nmm - 1))
                            k += 1
                    cv = Cp[:, ci * 512: ci * 512 + ni * 124].rearrange(
                        "p (i x) -> p i x", x=124)
                    # evict on ACT, then horizontal pool max (one PSUM operand)
                    nc.scalar.copy(Cs[:, i0:i0 + ni, :], cv)
                    nc.vector.tensor_max(Ch[:, i0:i0 + ni, :],
                                         Cs[:, i0:i0 + ni, 0:123],
                                         cv[:, :, 1:124])

                # vertical pool via partition-shift matmul; V = max(Ch,0,Sh)
                nc.gpsimd.dma_start(V[114:115, :, :], d["ones_v"][:])
                nc.gpsimd.dma_start(V[0:114, :, 0:1], d["zpad_v"][:, :, 0:1])
                nc.gpsimd.dma_start(V[0:114, :, 124:125], d["zpad_v"][:, :, 1:2])
                for ci, (i0, ni) in enumerate(SH_CHUNKS):
                    nc.tensor.matmul(
                        Sh[:, ci * 512: ci * 512 + ni * 123],
                        s1m_t[:], Ch[:, i0:i0 + ni, :],
                        start=True, stop=True)
                    sv = Sh[:, ci * 512: ci * 512 + ni * 123].rearrange(
                        "p (i x) -> p i x", x=123)
                    nc.vector.scalar_tensor_tensor(
                        V[0:114, i0:i0 + ni, 1:124],
                        Ch[0:114, i0:i0 + ni, :], 0.0, sv[0:114, :, :],
                        op0=ALU.max, op1=ALU.max)

            # ---- conv2 + pool2 ----
            C2s_a = sb.tile([120, B, 62], F32R)
            C2s_b = sb.tile([15, B, 62], F32R)
            C2h_a = sb.tile([120, B, 61], F32R)
            C2h_b = sb.tile([15, B, 61], F32R)
            V2 = sb.tile([120, B, NJ], F32)

            with tc.tile_pool(name="ps_2", bufs=1, space="PSUM") as ps2:
                C2a = ps2.tile([120, 1024], F32)
                C2b = ps2.tile([15, 1024], F32)
                Sh2 = ps2.tile([120, 1024], F32)

                for ci, (i0, ni) in enumerate(C2_CHUNKS):
                    for grp, (cp, m0, m1) in enumerate(
                            ((C2a, 0, 120), (C2b, 120, 135))):
                        for kxp in range(3):
                            rhs = V[0:115, i0:i0 + ni, kxp:kxp + 123:2]
                            nc.tensor.matmul(
                                cp[:, ci * 512: ci * 512 + ni * 62],
                                w2_t[:, kxp, m0:m1], rhs,
                                start=(kxp == 0), stop=(kxp == 2))
                    for cp, cs, ch in ((C2a, C2s_a, C2h_a), (C2b, C2s_b, C2h_b)):
                        cv = cp[:, ci * 512: ci * 512 + ni * 62].rearrange(
                            "p (i x) -> p i x", x=62)
                        nc.scalar.copy(cs[:, i0:i0 + ni, :], cv)
                        # relu + horizontal pool (one PSUM operand)
                        nc.vector.scalar_tensor_tensor(
                            ch[:, i0:i0 + ni, :],
                            cs[:, i0:i0 + ni, 0:61], 0.0, cv[:, :, 1:62],
                            op0=ALU.max, op1=ALU.max)

                for ci, (i0, ni) in enumerate(P2_CHUNKS):
                    nc.tensor.matmul(
                        Sh2[:, ci * 512: ci * 512 + ni * 61],
                        s2a_t[:], C2h_a[:, i0:i0 + ni, :],
                        start=True, stop=False)
                    nc.tensor.matmul(
                        Sh2[:, ci * 512: ci * 512 + ni * 61],
                        s2b_t[:], C2h_b[:, i0:i0 + ni, :],
                        start=False, stop=True)
                    sv = Sh2[:, ci * 512: ci * 512 + ni * 61].rearrange(
                        "p (i x) -> p i x", x=61)
                    nc.vector.tensor_max(V2[:, i0:i0 + ni, :],
                                         C2h_a[:, i0:i0 + ni, :], sv)

            # ---- fc1 (split-bf16, tensor-parallel contraction) ----
            h_hi = sb.tile([120, NJ, B], BF16)
            h_lo = sb.tile([120, NJ, B], BF16)
            v2t = V2[:].rearrange("p i j -> p j i")
            nc.vector.tensor_copy(h_hi[:], v2t)
            nc.vector.tensor_sub(h_lo[:], v2t, h_hi[:])

            fc1s = sb.tile([B, 120], F32)
            with tc.tile_pool(name="ps_3", bufs=1, space="PSUM") as ps3:
                fps = ps3.tile([B, 120], F32)
                for j in range(NJ):
                    nc.tensor.matmul(fps[:], h_hi[:, j, :], wslab_t[:, j, 0, :],
                                     start=(j == 0), stop=False)
                    nc.tensor.matmul(fps[:], h_lo[:, j, :], wslab_t[:, j, 0, :],
                                     start=False, stop=False)
                    nc.tensor.matmul(fps[:], h_hi[:, j, :], wslab_t[:, j, 1, :],
                                     start=False, stop=(j == NJ - 1))
                nc.vector.tensor_copy(fc1s[:], fps[:])

            # ---- AllReduce fc1 partials ----
            arin = dr.tile([B, 120], F32)
            arout = dr.tile([B, 120], F32, addr_space="Shared")
            nc.sync.dma_start(arin[:], fc1s[:])
            nc.gpsimd.collective_compute(
                "AllReduce", ALU.add,
                replica_groups=[list(range(N_CORES))],
                ins=[arin.opt()], outs=[arout.opt()])
            h1post = sb.tile([B, 120], F32)
            nc.sync.dma_start(h1post[:], arout[:])

            # ---- tail (replicated) ----
            h1 = sb.tile([120, B], F32)
            h2 = sb.tile([84, B], F32)
            h10 = sb.tile([B, 1], F32)
            s1 = sb.tile([20, 1], F32)
            fs_row = sb.tile([1, 5], F32)
            fsb = sb.tile([128, 5], F32)
            diff = sb.tile([128, 64, 5], F32)
            sq = sb.tile([128, 64, 5], F32)
            d2 = sb.tile([128, 64], F32)
            kxv = sb.tile([128, 64], F32)
            pr = sb.tile([128, 2, 64], F32)
            krw = sb.tile([128, 2], F32)
            ones_t = sb.tile([128, 1], F32)
            out_sb = sb.tile([1, 2], F32)
            nc.vector.memset(ones_t[:], 1.0)

            with tc.tile_pool(name="ps_4", bufs=1, space="PSUM") as ps4:
                tp = ps4.tile([120, B], F32)
                nc.tensor.transpose(tp[:], h1post[:], small["idt10"][:])
                nc.scalar.activation(h1[:], tp[:], AF.Relu,
                                     bias=small["fc1b"][:])

                p2 = ps4.tile([84, B], F32)
                nc.tensor.matmul(p2[:], small["w2fcT"][:], h1[:],
                                 start=True, stop=True)
                nc.scalar.activation(h2[:], p2[:], AF.Relu,
                                     bias=small["fc2b"][:])

                p3 = ps4.tile([B, 1], F32)
                nc.tensor.matmul(p3[:], h2[:], small["w3fcT"][:],
                                 start=True, stop=True)
                nc.scalar.activation(h10[:], p3[:], AF.Identity,
                                     bias=small["b3vec"][:])

                p4 = ps4.tile([20, 1], F32)
                nc.tensor.matmul(p4[:], small["wq1T"][:], h10[:],
                                 start=True, stop=True)
                nc.scalar.activation(s1[:], p4[:], AF.Tanh)

                p5 = ps4.tile([1, 5], F32)
                nc.tensor.matmul(p5[:], s1[:], small["wq2T"][:],
                                 start=True, stop=True)
                nc.scalar.activation(fs_row[:], p5[:], AF.Tanh)

                nc.gpsimd.partition_broadcast(fsb[:], fs_row[0:1, :])
                nc.vector.tensor_sub(
                    diff[:], small["ts_r"][:],
                    fsb[:].unsqueeze(1).broadcast_to([128, 64, 5]))
                nc.scalar.square(sq[:], diff[:])
                nc.vector.reduce_sum(d2[:], sq[:], axis=AX.X)
                nc.scalar.activation(kxv[:], d2[:], AF.Exp, scale=-1.0)
                nc.vector.tensor_mul(
                    pr[:], small["kcls_r"][:],
                    kxv[:].unsqueeze(1).broadcast_to([128, 2, 64]))
                nc.vector.reduce_sum(krw[:], pr[:], axis=AX.X)

                p6 = ps4.tile([1, 2], F32)
                nc.tensor.matmul(p6[:], ones_t[:], krw[:],
                                 start=True, stop=True)
                nc.vector.tensor_add(out_sb[:], p6[:], small["kclsb"][:])

            nc.sync.dma_start(out_d[:], out_sb[:])

    nc.compile()
    return nc


def _prep_inputs(inputs):
    f32 = np.float32
    x = np.asarray(inputs["x"], f32)
    conv1_w = np.asarray(inputs["conv1_w"], f32)
    conv1_b = np.asarray(inputs["conv1_b"], f32)
    conv2_w = np.asarray(inputs["conv2_w"], f32)
    conv2_b = np.asarray(inputs["conv2_b"], f32)
    fc1_w = np.asarray(inputs["fc1_w"], f32)
    fc1_b = np.asarray(inputs["fc1_b"], f32)
    fc2_w = np.asarray(inputs["fc2_w"], f32)
    fc2_b = np.asarray(inputs["fc2_b"], f32)
    fc3_w = np.asarray(inputs["fc3_w"], f32)
    fc3_b = np.asarray(inputs["fc3_b"], f32)
    qnn_w1 = np.asarray(inputs["qnn_w1"], f32)
    qnn_w2 = np.asarray(inputs["qnn_w2"], f32)
    ts = np.asarray(inputs["train_states"], f32)
    kcls_w = np.asarray(inputs["kcls_w"], f32)
    kcls_b = np.asarray(inputs["kcls_b"], f32)

    shared = {
        "fc1b": fc1_b.reshape(120, 1),
        "w2fcT": np.ascontiguousarray(fc2_w.T),
        "fc2b": fc2_b.reshape(84, 1),
        "w3fcT": np.ascontiguousarray(fc3_w.T),
        "b3vec": np.full((B, 1), fc3_b[0], f32),
        "wq1T": np.ascontiguousarray(qnn_w1.T),
        "wq2T": np.ascontiguousarray(qnn_w2.T),
        "idt10": np.eye(B, dtype=f32),
        "ts_r": np.ascontiguousarray(ts.reshape(128, 64, 5)),
        "kcls_r": np.ascontiguousarray(
            kcls_w.reshape(2, 128, 64).transpose(1, 0, 2)),
        "kclsb": kcls_b.reshape(1, 2),
    }

    fc1_w4 = fc1_w.reshape(120, 15, 61, 61)

    in_maps = []
    for a, b in BANDS:
        nb = b - a
        Y0 = 2 * a - 1          # conv1 row of y_loc 0 (also pool1 row of py_loc 0)
        X0 = 4 * a - 3          # x row of r_loc 0

        # x slab [3, XR, B, XC], zero-padded rows/cols
        xs = np.zeros((3, XR, B, XC), f32)
        r_lo = max(0, X0)
        r_hi = min(250, X0 + XR)
        xs[:, r_lo - X0: r_hi - X0, :, 1:251] = (
            x[:, :, r_lo:r_hi, :].transpose(1, 2, 0, 3))

        # conv1 banded weights: K=(c, r_loc)+bias, M=(y_loc, och), per kx
        w1 = np.zeros((3, 43, 5, 120), f32)     # [c, r_loc, kx, m=(y_loc,och)]
        for y_loc in range(C1R):
            y = Y0 + y_loc
            if not (0 <= y <= 123):
                continue
            for ky in range(5):
                r_loc = 2 * y_loc + ky
                if r_loc >= XR:
                    continue
                for c in range(3):
                    w1[c, r_loc, :, y_loc * 6: y_loc * 6 + 6] = \
                        conv1_w[:, c, ky, :].T
        w1a = np.zeros((87, 5, 120), f32)
        w1a[0:43] = w1[0]
        w1a[43:86] = w1[1]
        w1a[86, 0, :] = np.tile(conv1_b, C1R)   # bias row, kx=0 only
        w1b = np.ascontiguousarray(w1[2])

        # conv2 banded weights: K=(py_loc, ich)+bias@114, M=(i2_loc, och2)
        w2 = np.zeros((115, 3, 135), f32)
        for i2_loc in range(C2R):
            i2 = a + i2_loc
            if i2 > 61:
                continue
            for kyp in range(3):
                py_loc = 2 * i2_loc + kyp
                py = Y0 + py_loc
                if py_loc >= P1R or not (0 <= py <= 122):
                    continue
                for ich in range(6):
                    q = py_loc * 6 + ich
                    m0 = i2_loc * 15
                    w2[q, :, m0:m0 + 15] = conv2_w[:, ich, kyp, :].T
        w2[114, 0, :] = np.tile(conv2_b, 9)     # bias row, kxp=0 only

        # partition-shift matrices
        s1m = np.zeros((120, 114), f32)
        for m in range(114):
            s1m[m + 6, m] = 1.0
        s2a = np.zeros((120, 120), f32)
        s2b = np.zeros((15, 120), f32)
        for m in range(105):
            s2a[m + 15, m] = 1.0
        for m in range(105, 120):
            s2b[m - 105, m] = 1.0

        # fc1 weight slab, split bf16: [p=(i2_loc,och2), j, {hi,lo}, och1]
        wsl = np.zeros((8, 15, NJ, 120), f32)
        nrow = min(nb, 8)
        wsl[0:nrow] = fc1_w4[:, :, a:a + nrow, :].transpose(2, 1, 3, 0)
        wsl = wsl.reshape(120, NJ, 120)
        hi = wsl.astype(ml_dtypes.bfloat16)
        lo = (wsl - hi.astype(f32)).astype(ml_dtypes.bfloat16)
        wslab = np.stack([hi, lo], axis=2)      # [120, NJ, 2, 120] bf16

        m = dict(shared)
        m["ones_bx"] = np.ones((1, B, XC), f32)
        m["ones_v"] = np.ones((1, B, 125), f32)
        m["zpad_v"] = np.zeros((114, B, 2), f32)
        m.update({"x_slab": xs, "w1a": w1a, "w1b": w1b, "w2": w2,
                  "s1m": s1m, "s2a": s2a, "s2b": s2b,
                  "wslab": np.ascontiguousarray(wslab)})
        in_maps.append(m)
    return in_maps


_NC_CACHE = None


def kernel(**inputs) -> np.ndarray:
    global _NC_CACHE
    if _NC_CACHE is None:
        _NC_CACHE = _build_nc()
    nc = _NC_CACHE
    in_maps = _prep_inputs(inputs)
    res = bass_utils.run_bass_kernel_spmd(
        nc, in_maps, core_ids=list(range(N_CORES)))
    return res.results[0]["out"]


# revision 11
# speedup vs baseline: 1.2898x; 1.1726x over previous
"""Trainium2 Bass kernel for nn_ClassicalHybridClassifier.

Pipeline: conv1(5x5,s2) -> maxpool(2,s1) -> conv2(3x3,s2) -> maxpool(2,s1)
          -> fc1 [120,55815] -> fc2 -> fc3 -> qnn tanh stack -> RBF vs 8192
          train states -> [1,2] output.

Sharding: each of the 8 cores computes a horizontal band of the conv pipeline
(bands over the 61 pool2 output rows: 8,8,8,8,8,7,7,7) and the matching
contraction slice of fc1 (tensor-parallel over fc1's 55815 input dim, weights
restructured host-side to match the on-chip feature layout). One AllReduce of
the [120,10] fc1 partials; the tiny tail (fc2/fc3/qnn/RBF over all 8192 train
states) is replicated on every core.

Convs are expressed as banded-weight matmuls: contraction over (channel,
input row) with the 5 (resp. 3) kernel-column taps accumulated in PSUM via
column-shifted strided views of the input rows. Vertical max-pools cross the
partition dim, handled by a partition-shift matmul. fc1 runs as 61 per-column
matmul triples in split-bf16 (hi/lo) for ~fp32 accuracy at bf16 speed.
"""

import numpy as np
import ml_dtypes

import concourse.bass as bass
import concourse.mybir as mybir
import concourse.tile as tile
from concourse import bass_utils, bacc

F32 = mybir.dt.float32
F32R = mybir.dt.float32r
BF16 = mybir.dt.bfloat16
AF = mybir.ActivationFunctionType
ALU = mybir.AluOpType
AX = mybir.AxisListType

N_CORES = 8
BANDS = [(0, 8), (8, 16), (16, 24), (24, 32), (32, 40), (40, 47), (47, 54), (54, 61)]

B = 10          # batch
XR = 43         # x rows per core (padded)
XC = 252        # x cols incl 1+1 zero pad
C1R = 20        # conv1 out rows per core (padded)
P1R = 19        # pool1 rows per core (padded)
C2R = 9         # conv2 out rows per core (padded)
NJ = 61         # pool2 / fc1 spatial columns
C1CH = 6
C2CH = 15

# conv1 N chunking over images (PSUM bank = 512 fp32)
C1_CHUNKS = [(0, 4), (4, 3), (7, 3)]     # (img0, nimg): 4*124=496, 3*124=372
C2_CHUNKS = [(0, 8), (8, 2)]             # 8*62=496, 2*62=124
P2_CHUNKS = [(0, 8), (8, 2)]             # over (img, 61): 488, 122
SH_CHUNKS = [(0, 4), (4, 4), (8, 2)]     # shift-mm chunks: even N (492, 492, 246)


def _build_nc():
    nc = bacc.Bacc("TRN2", target_bir_lowering=False, debug=False,
                   num_devices=N_CORES)

    d = {}
    def din(name, shape, dt):
        d[name] = nc.dram_tensor(name, list(shape), dt, kind="ExternalInput").ap()

    din("x2", (87, B, XC), F32R)       # c0 rows, c1 rows, ones row
    din("x3", (43, B, XC), F32R)       # c2 rows
    din("w1a", (87, 5, 120), F32R)
    din("w1b", (43, 5, 120), F32R)
    din("pack2", (120, 768), F32R)     # s1m | s2a | s2b | w2
    din("pack1", (128, 576), F32)      # small fc/tail tensors
    din("wslab", (120, NJ, 2, 120), BF16)
    din("ones_v", (1, B, 125), F32R)
    din("zpad_v", (114, B, 2), F32R)

    out_d = nc.dram_tensor("out", [1, 2], F32, kind="ExternalOutput").ap()
    warm_d = nc.dram_tensor("warm", [1, 4], F32, kind="ExternalOutput").ap()

    with tile.TileContext(nc) as tc:
        with (
            tc.tile_pool(name="sb", bufs=1) as sb,
            tc.tile_pool(name="dr", bufs=1, space="DRAM") as dr,
        ):
            # ---- DMAs in (big transfers on SWDGE, priority order) ----
            x_a = sb.tile([87, B, XC], F32R)       # (c0,c1 rows) + ones row 86
            x_b = sb.tile([43, B, XC], F32R)       # c2 rows
            w1a_t = sb.tile([87, 5, 120], F32R)
            w1b_t = sb.tile([43, 5, 120], F32R)
            pack2_t = sb.tile([120, 768], F32R)
            pack1_t = sb.tile([128, 576], F32)
            wslab_t = sb.tile([120, NJ, 2, 120], BF16)
            nc.gpsimd.dma_start(x_a[:], d["x2"][:])
            nc.gpsimd.dma_start(x_b[:], d["x3"][:])
            nc.gpsimd.dma_start(w1a_t[:], d["w1a"][:])
            nc.gpsimd.dma_start(w1b_t[:], d["w1b"][:])
            nc.gpsimd.dma_start(pack2_t[:], d["pack2"][:])
            nc.gpsimd.dma_start(pack1_t[:], d["pack1"][:])
            nc.gpsimd.dma_start(wslab_t[:], d["wslab"][:])

            s1m_t = pack2_t[0:120, 0:114]
            s2a_t = pack2_t[0:120, 114:234]
            s2b_t = pack2_t[0:15, 234:354]
            w2f = pack2_t[0:115, 354:759]          # [115, 3*135] flat

            small = {
                "fc1b": pack1_t[0:120, 0:1],
                "w2fcT": pack1_t[0:120, 1:85],
                "fc2b": pack1_t[0:84, 85:86],
                "w3fcT": pack1_t[0:84, 86:87],
                "b3vec": pack1_t[0:B, 87:88],
                "wq1T": pack1_t[0:B, 88:108],
                "wq2T": pack1_t[0:20, 108:113],
                "idt10": pack1_t[0:B, 113:123],
                "kclsb": pack1_t[0:1, 123:125],
                "ts_r": pack1_t[:, 128:448].rearrange("p (a b) -> p a b", b=5),
                "kcls_r": pack1_t[:, 448:576].rearrange("p (a b) -> p a b", b=64),
            }

            # ---- PE warmup during input DMA ----
            with tc.tile_pool(name="ps_w", bufs=1, space="PSUM") as ps_w:
                wsc = sb.tile([128, 512], BF16)
                nc.vector.memset(wsc[:], 0.0)
                wps = ps_w.tile([128, 512], F32)
                for i in range(10):
                    nc.tensor.matmul(wps[:, 0:512], wsc[:, 0:128], wsc[:],
                                     start=(i == 0), stop=(i == 9))
                wout = sb.tile([1, 4], F32)
                nc.vector.tensor_copy(wout[:], wps[0:1, 0:4])
                nc.sync.dma_start(warm_d[:], wout[:])

            # ---- conv1 + pool1 ----
            Cs = sb.tile([120, B, 124], F32R)      # conv1 psum eviction
            Ch = sb.tile([120, B, 123], F32R)      # horizontal max
            V = sb.tile([128, B, 125], F32R)       # pool1 out, (py,ich) + ones@114

            with tc.tile_pool(name="ps_1", bufs=1, space="PSUM") as ps1:
                Cp = ps1.tile([120, 1536], F32)    # conv1 psum, 3 banks
                Sh = ps1.tile([114, 1536], F32)    # shifted Ch

                for ci, (i0, ni) in enumerate(C1_CHUNKS):
                    nmm = 10
                    k = 0
                    for kx in range(5):
                        for grp in range(2):
                            xt, wt, kdim = ((x_a, w1a_t, 87) if grp == 0
                                            else (x_b, w1b_t, 43))
                            rhs = xt[0:kdim, i0:i0 + ni, kx:kx + 248:2]
                            nc.tensor.matmul(
                                Cp[:, ci * 512: ci * 512 + ni * 124],
                                wt[0:kdim, kx, :], rhs,
                                start=(k == 0), stop=(k == nmm - 1))
                            k += 1
                    cv = Cp[:, ci * 512: ci * 512 + ni * 124].rearrange(
                        "p (i x) -> p i x", x=124)
                    # evict on ACT, then horizontal pool max (one PSUM operand)
                    nc.scalar.copy(Cs[:, i0:i0 + ni, :], cv)
                    nc.vector.tensor_max(Ch[:, i0:i0 + ni, :],
                                         Cs[:, i0:i0 + ni, 0:123],
                                         cv[:, :, 1:124])

                # vertical pool via partition-shift matmul; V = max(Ch,0,Sh)
                nc.sync.dma_start(V[114:115, :, :], d["ones_v"][:])
                nc.sync.dma_start(V[0:114, :, 0:1], d["zpad_v"][:, :, 0:1])
                nc.sync.dma_start(V[0:114, :, 124:125], d["zpad_v"][:, :, 1:2])
                for ci, (i0, ni) in enumerate(SH_CHUNKS):
                    nc.tensor.matmul(
                        Sh[:, ci * 512: ci * 512 + ni * 123],
                        s1m_t[:], Ch[:, i0:i0 + ni, :],
                        start=True, stop=True)
                    sv = Sh[:, ci * 512: ci * 512 + ni * 123].rearrange(
                        "p (i x) -> p i x", x=123)
                    nc.vector.scalar_tensor_tensor(
                        V[0:114, i0:i0 + ni, 1:124],
                        Ch[0:114, i0:i0 + ni, :], 0.0, sv[0:114, :, :],
                        op0=ALU.max, op1=ALU.max)

            # ---- conv2 + pool2 ----
            C2s_a = sb.tile([120, B, 62], F32R)
            C2s_b = sb.tile([15, B, 62], F32R)
            C2h_a = sb.tile([120, B, 61], F32R)
            C2h_b = sb.tile([15, B, 61], F32R)
            V2 = sb.tile([120, B, NJ], F32)

            with tc.tile_pool(name="ps_2", bufs=1, space="PSUM") as ps2:
                C2a = ps2.tile([120, 1024], F32)
                C2b = ps2.tile([15, 1024], F32)
                Sh2 = ps2.tile([120, 1024], F32)

                for ci, (i0, ni) in enumerate(C2_CHUNKS):
                    for grp, (cp, m0, m1) in enumerate(
                            ((C2a, 0, 120), (C2b, 120, 135))):
                        for kxp in range(3):
                            rhs = V[0:115, i0:i0 + ni, kxp:kxp + 123:2]
                            nc.tensor.matmul(
                                cp[:, ci * 512: ci * 512 + ni * 62],
                                w2f[:, kxp * 135 + m0: kxp * 135 + m1], rhs,
                                start=(kxp == 0), stop=(kxp == 2))
                    for cp, cs, ch in ((C2a, C2s_a, C2h_a), (C2b, C2s_b, C2h_b)):
                        cv = cp[:, ci * 512: ci * 512 + ni * 62].rearrange(
                            "p (i x) -> p i x", x=62)
                        nc.scalar.copy(cs[:, i0:i0 + ni, :], cv)
                        # relu + horizontal pool (one PSUM operand)
                        nc.vector.scalar_tensor_tensor(
                            ch[:, i0:i0 + ni, :],
                            cs[:, i0:i0 + ni, 0:61], 0.0, cv[:, :, 1:62],
                            op0=ALU.max, op1=ALU.max)

                for ci, (i0, ni) in enumerate(P2_CHUNKS):
                    nc.tensor.matmul(
                        Sh2[:, ci * 512: ci * 512 + ni * 61],
                        s2a_t[:], C2h_a[:, i0:i0 + ni, :],
                        start=True, stop=False)
                    nc.tensor.matmul(
                        Sh2[:, ci * 512: ci * 512 + ni * 61],
                        s2b_t[:], C2h_b[:, i0:i0 + ni, :],
                        start=False, stop=True)
                    sv = Sh2[:, ci * 512: ci * 512 + ni * 61].rearrange(
                        "p (i x) -> p i x", x=61)
                    nc.vector.tensor_max(V2[:, i0:i0 + ni, :],
                                         C2h_a[:, i0:i0 + ni, :], sv)

            # ---- fc1 (split-bf16, tensor-parallel contraction) ----
            h_hi = sb.tile([120, NJ, B], BF16)
            h_lo = sb.tile([120, NJ, B], BF16)
            v2t = V2[:].rearrange("p i j -> p j i")
            nc.vector.tensor_copy(h_hi[:], v2t)
            nc.vector.tensor_sub(h_lo[:], v2t, h_hi[:])

            fc1s = sb.tile([B, 120], F32)
            with tc.tile_pool(name="ps_3", bufs=1, space="PSUM") as ps3:
                fps = ps3.tile([B, 120], F32)
                for j in range(NJ):
                    nc.tensor.matmul(fps[:], h_hi[:, j, :], wslab_t[:, j, 0, :],
                                     start=(j == 0), stop=False)
                    nc.tensor.matmul(fps[:], h_lo[:, j, :], wslab_t[:, j, 0, :],
                                     start=False, stop=False)
                    nc.tensor.matmul(fps[:], h_hi[:, j, :], wslab_t[:, j, 1, :],
                                     start=False, stop=(j == NJ - 1))
                nc.vector.tensor_copy(fc1s[:], fps[:])

            # ---- AllReduce fc1 partials ----
            arin = dr.tile([B, 120], F32)
            arout = dr.tile([B, 120], F32, addr_space="Shared")
            nc.sync.dma_start(arin[:], fc1s[:])
            nc.gpsimd.collective_compute(
                "AllReduce", ALU.add,
                replica_groups=[list(range(N_CORES))],
                ins=[arin.opt()], outs=[arout.opt()])
            h1post = sb.tile([B, 120], F32)
            nc.sync.dma_start(h1post[:], arout[:])

            # ---- tail (replicated) ----
            h1 = sb.tile([120, B], F32)
            h2 = sb.tile([84, B], F32)
            h10 = sb.tile([B, 1], F32)
            s1 = sb.tile([20, 1], F32)
            fs_row = sb.tile([1, 5], F32)
            fsb = sb.tile([128, 5], F32)
            diff = sb.tile([128, 64, 5], F32)
            sq = sb.tile([128, 64, 5], F32)
            d2 = sb.tile([128, 64], F32)
            kxv = sb.tile([128, 64], F32)
            pr = sb.tile([128, 2, 64], F32)
            krw = sb.tile([128, 2], F32)
            ones_t = sb.tile([128, 1], F32)
            out_sb = sb.tile([1, 2], F32)
            nc.vector.memset(ones_t[:], 1.0)

            with tc.tile_pool(name="ps_4", bufs=1, space="PSUM") as ps4:
                tp = ps4.tile([120, B], F32)
                nc.tensor.transpose(tp[:], h1post[:], small["idt10"][:])
                nc.scalar.activation(h1[:], tp[:], AF.Relu,
                                     bias=small["fc1b"][:])

                p2 = ps4.tile([84, B], F32)
                nc.tensor.matmul(p2[:], small["w2fcT"][:], h1[:],
                                 start=True, stop=True)
                nc.scalar.activation(h2[:], p2[:], AF.Relu,
                                     bias=small["fc2b"][:])

                p3 = ps4.tile([B, 1], F32)
                nc.tensor.matmul(p3[:], h2[:], small["w3fcT"][:],
                                 start=True, stop=True)
                nc.scalar.activation(h10[:], p3[:], AF.Identity,
                                     bias=small["b3vec"][:])

                p4 = ps4.tile([20, 1], F32)
                nc.tensor.matmul(p4[:], small["wq1T"][:], h10[:],
                                 start=True, stop=True)
                nc.scalar.activation(s1[:], p4[:], AF.Tanh)

                p5 = ps4.tile([1, 5], F32)
                nc.tensor.matmul(p5[:], s1[:], small["wq2T"][:],
                                 start=True, stop=True)
                nc.scalar.activation(fs_row[:], p5[:], AF.Tanh)

                nc.gpsimd.partition_broadcast(fsb[:], fs_row[0:1, :])
                nc.vector.tensor_sub(
                    diff[:], small["ts_r"][:],
                    fsb[:].unsqueeze(1).broadcast_to([128, 64, 5]))
                nc.scalar.square(sq[:], diff[:])
                nc.vector.reduce_sum(d2[:], sq[:], axis=AX.X)
                nc.scalar.activation(kxv[:], d2[:], AF.Exp, scale=-1.0)
                nc.vector.tensor_mul(
                    pr[:], small["kcls_r"][:],
                    kxv[:].unsqueeze(1).broadcast_to([128, 2, 64]))
                nc.vector.reduce_sum(krw[:], pr[:], axis=AX.X)

                p6 = ps4.tile([1, 2], F32)
                nc.tensor.matmul(p6[:], ones_t[:], krw[:],
                                 start=True, stop=True)
                nc.vector.tensor_add(out_sb[:], p6[:], small["kclsb"][:])

            nc.sync.dma_start(out_d[:], out_sb[:])

    nc.compile()
    return nc


def _prep_inputs(inputs):
    f32 = np.float32
    x = np.asarray(inputs["x"], f32)
    conv1_w = np.asarray(inputs["conv1_w"], f32)
    conv1_b = np.asarray(inputs["conv1_b"], f32)
    conv2_w = np.asarray(inputs["conv2_w"], f32)
    conv2_b = np.asarray(inputs["conv2_b"], f32)
    fc1_w = np.asarray(inputs["fc1_w"], f32)
    fc1_b = np.asarray(inputs["fc1_b"], f32)
    fc2_w = np.asarray(inputs["fc2_w"], f32)
    fc2_b = np.asarray(inputs["fc2_b"], f32)
    fc3_w = np.asarray(inputs["fc3_w"], f32)
    fc3_b = np.asarray(inputs["fc3_b"], f32)
    qnn_w1 = np.asarray(inputs["qnn_w1"], f32)
    qnn_w2 = np.asarray(inputs["qnn_w2"], f32)
    ts = np.asarray(inputs["train_states"], f32)
    kcls_w = np.asarray(inputs["kcls_w"], f32)
    kcls_b = np.asarray(inputs["kcls_b"], f32)

    pack1 = np.zeros((128, 576), f32)
    pack1[0:120, 0:1] = fc1_b.reshape(120, 1)
    pack1[0:120, 1:85] = fc2_w.T
    pack1[0:84, 85:86] = fc2_b.reshape(84, 1)
    pack1[0:84, 86:87] = fc3_w.T
    pack1[0:B, 87:88] = fc3_b[0]
    pack1[0:B, 88:108] = qnn_w1.T
    pack1[0:20, 108:113] = qnn_w2.T
    pack1[0:B, 113:123] = np.eye(B, dtype=f32)
    pack1[0:1, 123:125] = kcls_b.reshape(1, 2)
    pack1[:, 128:448] = ts.reshape(128, 320)
    pack1[:, 448:576] = kcls_w.reshape(2, 128, 64).transpose(1, 0, 2).reshape(128, 128)
    shared = {"pack1": pack1}

    fc1_w4 = fc1_w.reshape(120, 15, 61, 61)

    in_maps = []
    for a, b in BANDS:
        nb = b - a
        Y0 = 2 * a - 1          # conv1 row of y_loc 0 (also pool1 row of py_loc 0)
        X0 = 4 * a - 3          # x row of r_loc 0

        # x slabs: x2 = [c0 rows | c1 rows | ones], x3 = [c2 rows]
        xs = np.zeros((3, XR, B, XC), f32)
        r_lo = max(0, X0)
        r_hi = min(250, X0 + XR)
        xs[:, r_lo - X0: r_hi - X0, :, 1:251] = (
            x[:, :, r_lo:r_hi, :].transpose(1, 2, 0, 3))
        x2 = np.concatenate(
            [xs[0], xs[1], np.ones((1, B, XC), f32)], axis=0)
        x3 = xs[2]

        # conv1 banded weights: K=(c, r_loc)+bias, M=(y_loc, och), per kx
        w1 = np.zeros((3, 43, 5, 120), f32)     # [c, r_loc, kx, m=(y_loc,och)]
        for y_loc in range(C1R):
            y = Y0 + y_loc
            if not (0 <= y <= 123):
                continue
            for ky in range(5):
                r_loc = 2 * y_loc + ky
                if r_loc >= XR:
                    continue
                for c in range(3):
                    w1[c, r_loc, :, y_loc * 6: y_loc * 6 + 6] = \
                        conv1_w[:, c, ky, :].T
        w1a = np.zeros((87, 5, 120), f32)
        w1a[0:43] = w1[0]
        w1a[43:86] = w1[1]
        w1a[86, 0, :] = np.tile(conv1_b, C1R)   # bias row, kx=0 only
        w1b = np.ascontiguousarray(w1[2])

        # conv2 banded weights: K=(py_loc, ich)+bias@114, M=(i2_loc, och2)
        w2 = np.zeros((115, 3, 135), f32)
        for i2_loc in range(C2R):
            i2 = a + i2_loc
            if i2 > 61:
                continue
            for kyp in range(3):
                py_loc = 2 * i2_loc + kyp
                py = Y0 + py_loc
                if py_loc >= P1R or not (0 <= py <= 122):
                    continue
                for ich in range(6):
                    q = py_loc * 6 + ich
                    m0 = i2_loc * 15
                    w2[q, :, m0:m0 + 15] = conv2_w[:, ich, kyp, :].T
        w2[114, 0, :] = np.tile(conv2_b, 9)     # bias row, kxp=0 only

        # partition-shift matrices
        s1m = np.zeros((120, 114), f32)
        for m in range(114):
            s1m[m + 6, m] = 1.0
        s2a = np.zeros((120, 120), f32)
        s2b = np.zeros((15, 120), f32)
        for m in range(105):
            s2a[m + 15, m] = 1.0
        for m in range(105, 120):
            s2b[m - 105, m] = 1.0

        # fc1 weight slab, split bf16: [p=(i2_loc,och2), j, {hi,lo}, och1]
        wsl = np.zeros((8, 15, NJ, 120), f32)
        nrow = min(nb, 8)
        wsl[0:nrow] = fc1_w4[:, :, a:a + nrow, :].transpose(2, 1, 3, 0)
        wsl = wsl.reshape(120, NJ, 120)
        hi = wsl.astype(ml_dtypes.bfloat16)
        lo = (wsl - hi.astype(f32)).astype(ml_dtypes.bfloat16)
        wslab = np.stack([hi, lo], axis=2)      # [120, NJ, 2, 120] bf16

        pack2 = np.zeros((120, 768), f32)
        pack2[0:120, 0:114] = s1m
        pack2[0:120, 114:234] = s2a
        pack2[0:15, 234:354] = s2b
        pack2[0:115, 354:759] = w2.reshape(115, 405)

        m = dict(shared)
        m["ones_v"] = np.ones((1, B, 125), f32)
        m["zpad_v"] = np.zeros((114, B, 2), f32)
        m.update({"x2": x2, "x3": x3, "w1a": w1a, "w1b": w1b,
                  "pack2": pack2, "wslab": np.ascontiguousarray(wslab)})
        in_maps.append(m)
    return in_maps


_NC_CACHE = None


def kernel(**inputs) -> np.ndarray:
    global _NC_CACHE
    if _NC_CACHE is None:
        _NC_CACHE = _build_nc()
    nc = _NC_CACHE
    in_maps = _prep_inputs(inputs)
    res = bass_utils.run_bass_kernel_spmd(
        nc, in_maps, core_ids=list(range(N_CORES)))
    return res.results[0]["out"]


# revision 12
# speedup vs baseline: 1.4126x; 1.0952x over previous
"""Trainium2 Bass kernel for nn_ClassicalHybridClassifier.

Pipeline: conv1(5x5,s2) -> maxpool(2,s1) -> conv2(3x3,s2) -> maxpool(2,s1)
          -> fc1 [120,55815] -> fc2 -> fc3 -> qnn tanh stack -> RBF vs 8192
          train states -> [1,2] output.

Sharding: each of the 8 cores computes a horizontal band of the conv pipeline
(bands over the 61 pool2 output rows: 8,8,8,8,8,7,7,7) and the matching
contraction slice of fc1 (tensor-parallel over fc1's 55815 input dim, weights
restructured host-side to match the on-chip feature layout). One AllReduce of
the [120,10] fc1 partials; the tiny tail (fc2/fc3/qnn/RBF over all 8192 train
states) is replicated on every core.

Convs are expressed as banded-weight matmuls: contraction over (channel,
input row) with the 5 (resp. 3) kernel-column taps accumulated in PSUM via
column-shifted strided views of the input rows. Vertical max-pools cross the
partition dim, handled by a partition-shift matmul. fc1 runs as 61 per-column
matmul triples in split-bf16 (hi/lo) for ~fp32 accuracy at bf16 speed.
"""

import numpy as np
import ml_dtypes

import concourse.bass as bass
import concourse.mybir as mybir
import concourse.tile as tile
from concourse import bass_utils, bacc

F32 = mybir.dt.float32
F32R = mybir.dt.float32r
BF16 = mybir.dt.bfloat16
AF = mybir.ActivationFunctionType
ALU = mybir.AluOpType
AX = mybir.AxisListType

N_CORES = 8
BANDS = [(0, 8), (8, 16), (16, 24), (24, 32), (32, 40), (40, 47), (47, 54), (54, 61)]

B = 10          # batch
XR = 43         # x rows per core (padded)
XC = 252        # x cols incl 1+1 zero pad
C1R = 20        # conv1 out rows per core (padded)
P1R = 19        # pool1 rows per core (padded)
C2R = 9         # conv2 out rows per core (padded)
NJ = 61         # pool2 / fc1 spatial columns
C1CH = 6
C2CH = 15

# conv1 N chunking over images (PSUM bank = 512 fp32)
C1_CHUNKS = [(0, 4), (4, 3), (7, 3)]     # (img0, nimg): 4*124=496, 3*124=372
C2_CHUNKS = [(0, 8), (8, 2)]             # 8*62=496, 2*62=124
P2_CHUNKS = [(0, 8), (8, 2)]             # over (img, 61): 488, 122
SH_CHUNKS = [(0, 4), (4, 4), (8, 2)]     # shift-mm chunks: even N (492, 492, 246)


def _build_nc():
    nc = bacc.Bacc("TRN2", target_bir_lowering=False, debug=False,
                   num_devices=N_CORES)

    d = {}
    def din(name, shape, dt):
        d[name] = nc.dram_tensor(name, list(shape), dt, kind="ExternalInput").ap()

    din("x2", (87, B, XC), F32R)       # c0 rows, c1 rows, ones row
    din("x3", (43, B, XC), F32R)       # c2 rows
    din("w1a", (87, 5, 120), F32R)
    din("w1b", (43, 5, 120), F32R)
    din("pack2", (120, 768), F32R)     # s1m | s2a | s2b | w2
    din("pack1", (128, 576), F32)      # small fc/tail tensors
    din("wslab", (120, NJ, 2, 120), BF16)
    din("ones_v", (1, B, 125), F32R)
    din("zpad_v", (114, B, 2), F32R)

    out_d = nc.dram_tensor("out", [1, 2], F32, kind="ExternalOutput").ap()
    warm_d = nc.dram_tensor("warm", [1, 4], F32, kind="ExternalOutput").ap()

    with tile.TileContext(nc) as tc:
        with (
            tc.tile_pool(name="sb", bufs=1) as sb,
            tc.tile_pool(name="dr", bufs=1, space="DRAM") as dr,
        ):
            # ---- DMAs in (big transfers on SWDGE, priority order) ----
            x_a = sb.tile([87, B, XC], F32R)       # (c0,c1 rows) + ones row 86
            x_b = sb.tile([43, B, XC], F32R)       # c2 rows
            w1a_t = sb.tile([87, 5, 120], F32R)
            w1b_t = sb.tile([43, 5, 120], F32R)
            pack2_t = sb.tile([120, 768], F32R)
            pack1_t = sb.tile([128, 576], F32)
            wslab_t = sb.tile([120, NJ, 2, 120], BF16)
            nc.gpsimd.dma_start(x_a[:], d["x2"][:])
            nc.gpsimd.dma_start(x_b[:], d["x3"][:])
            nc.gpsimd.dma_start(w1a_t[:], d["w1a"][:])
            nc.gpsimd.dma_start(w1b_t[:], d["w1b"][:])
            nc.gpsimd.dma_start(pack2_t[:], d["pack2"][:])
            nc.gpsimd.dma_start(pack1_t[:], d["pack1"][:])
            nc.gpsimd.dma_start(wslab_t[:], d["wslab"][:])

            s1m_t = pack2_t[0:120, 0:114]
            s2a_t = pack2_t[0:120, 114:234]
            s2b_t = pack2_t[0:15, 234:354]
            w2f = pack2_t[0:115, 354:759]          # [115, 3*135] flat

            small = {
                "fc1b": pack1_t[0:120, 0:1],
                "w2fcT": pack1_t[0:120, 1:85],
                "fc2b": pack1_t[0:84, 85:86],
                "w3fcT": pack1_t[0:84, 86:87],
                "b3vec": pack1_t[0:B, 87:88],
                "wq1T": pack1_t[0:B, 88:108],
                "wq2T": pack1_t[0:20, 108:113],
                "idt10": pack1_t[0:B, 113:123],
                "kclsb": pack1_t[0:1, 123:125],
                "ts_r": pack1_t[:, 128:448].rearrange("p (a b) -> p a b", b=5),
                "kcls_r": pack1_t[:, 448:576].rearrange("p (a b) -> p a b", b=64),
            }

            # ---- PE warmup during input DMA ----
            with tc.tile_pool(name="ps_w", bufs=1, space="PSUM") as ps_w:
                wsc = sb.tile([128, 512], BF16)
                nc.vector.memset(wsc[:], 0.0)
                wps = ps_w.tile([128, 512], F32)
                for i in range(10):
                    nc.tensor.matmul(wps[:, 0:512], wsc[:, 0:128], wsc[:],
                                     start=(i == 0), stop=(i == 9))
                wout = sb.tile([1, 4], F32)
                nc.vector.tensor_copy(wout[:], wps[0:1, 0:4])
                nc.sync.dma_start(warm_d[:], wout[:])

            # ---- conv1 + pool1 ----
            Cs = sb.tile([120, B, 124], F32R)      # conv1 psum eviction
            Ch = sb.tile([120, B, 123], F32R)      # horizontal max
            V = sb.tile([128, B, 125], F32R)       # pool1 out, (py,ich) + ones@114

            with tc.tile_pool(name="ps_1", bufs=1, space="PSUM") as ps1:
                Cp = ps1.tile([120, 1536], F32)    # conv1 psum, 3 banks
                Sh = ps1.tile([114, 1536], F32)    # shifted Ch

                for ci, (i0, ni) in enumerate(C1_CHUNKS):
                    nmm = 10
                    k = 0
                    for kx in range(5):
                        for grp in range(2):
                            xt, wt, kdim = ((x_a, w1a_t, 87) if grp == 0
                                            else (x_b, w1b_t, 43))
                            rhs = xt[0:kdim, i0:i0 + ni, kx:kx + 248:2]
                            nc.tensor.matmul(
                                Cp[:, ci * 512: ci * 512 + ni * 124],
                                wt[0:kdim, kx, :], rhs,
                                start=(k == 0), stop=(k == nmm - 1))
                            k += 1
                    cv = Cp[:, ci * 512: ci * 512 + ni * 124].rearrange(
                        "p (i x) -> p i x", x=124)
                    # evict on ACT, then horizontal pool max (one PSUM operand)
                    nc.scalar.copy(Cs[:, i0:i0 + ni, :], cv)
                    nc.vector.tensor_max(Ch[:, i0:i0 + ni, :],
                                         Cs[:, i0:i0 + ni, 0:123],
                                         cv[:, :, 1:124])

                # vertical pool via partition-shift matmul; V = max(Ch,0,Sh)
                nc.sync.dma_start(V[114:115, :, :], d["ones_v"][:])
                nc.vector.memset(V[0:114, :, 0:1].bitcast(F32), 0.0)
                nc.vector.memset(V[0:114, :, 124:125].bitcast(F32), 0.0)
                for ci, (i0, ni) in enumerate(SH_CHUNKS):
                    nc.tensor.matmul(
                        Sh[:, ci * 512: ci * 512 + ni * 123],
                        s1m_t[:], Ch[:, i0:i0 + ni, :],
                        start=True, stop=True)
                    sv = Sh[:, ci * 512: ci * 512 + ni * 123].rearrange(
                        "p (i x) -> p i x", x=123)
                    nc.vector.scalar_tensor_tensor(
                        V[0:114, i0:i0 + ni, 1:124],
                        Ch[0:114, i0:i0 + ni, :], 0.0, sv[0:114, :, :],
                        op0=ALU.max, op1=ALU.max)

            # ---- conv2 + pool2 ----
            C2s_a = sb.tile([120, B, 62], F32R)
            C2s_b = sb.tile([15, B, 62], F32R)
            C2h_a = sb.tile([120, B, 61], F32R)
            C2h_b = sb.tile([15, B, 61], F32R)
            V2 = sb.tile([120, B, NJ], F32)

            with tc.tile_pool(name="ps_2", bufs=1, space="PSUM") as ps2:
                C2a = ps2.tile([120, 1024], F32)
                C2b = ps2.tile([15, 1024], F32)
                Sh2 = ps2.tile([120, 1024], F32)

                for ci, (i0, ni) in enumerate(C2_CHUNKS):
                    for grp, (cp, m0, m1) in enumerate(
                            ((C2a, 0, 120), (C2b, 120, 135))):
                        for kxp in range(3):
                            rhs = V[0:115, i0:i0 + ni, kxp:kxp + 123:2]
                            nc.tensor.matmul(
                                cp[:, ci * 512: ci * 512 + ni * 62],
                                w2f[:, kxp * 135 + m0: kxp * 135 + m1], rhs,
                                start=(kxp == 0), stop=(kxp == 2))
                    for cp, cs, ch in ((C2a, C2s_a, C2h_a), (C2b, C2s_b, C2h_b)):
                        cv = cp[:, ci * 512: ci * 512 + ni * 62].rearrange(
                            "p (i x) -> p i x", x=62)
                        nc.scalar.copy(cs[:, i0:i0 + ni, :], cv)
                        # relu + horizontal pool (one PSUM operand)
                        nc.vector.scalar_tensor_tensor(
                            ch[:, i0:i0 + ni, :],
                            cs[:, i0:i0 + ni, 0:61], 0.0, cv[:, :, 1:62],
                            op0=ALU.max, op1=ALU.max)

                for ci, (i0, ni) in enumerate(P2_CHUNKS):
                    nc.tensor.matmul(
                        Sh2[:, ci * 512: ci * 512 + ni * 61],
                        s2a_t[:], C2h_a[:, i0:i0 + ni, :],
                        start=True, stop=False)
                    nc.tensor.matmul(
                        Sh2[:, ci * 512: ci * 512 + ni * 61],
                        s2b_t[:], C2h_b[:, i0:i0 + ni, :],
                        start=False, stop=True)
                    sv = Sh2[:, ci * 512: ci * 512 + ni * 61].rearrange(
                        "p (i x) -> p i x", x=61)
                    nc.vector.tensor_max(V2[:, i0:i0 + ni, :],
                                         C2h_a[:, i0:i0 + ni, :], sv)

            # ---- fc1 (split-bf16, tensor-parallel contraction) ----
            h_hi = sb.tile([120, NJ, B], BF16)
            h_lo = sb.tile([120, NJ, B], BF16)
            v2t = V2[:].rearrange("p i j -> p j i")
            nc.vector.tensor_copy(h_hi[:], v2t)
            nc.vector.tensor_sub(h_lo[:], v2t, h_hi[:])

            fc1s = sb.tile([B, 120], F32)
            with tc.tile_pool(name="ps_3", bufs=1, space="PSUM") as ps3:
                fps = ps3.tile([B, 120], F32)
                for j in range(NJ):
                    nc.tensor.matmul(fps[:], h_hi[:, j, :], wslab_t[:, j, 0, :],
                                     start=(j == 0), stop=False)
                    nc.tensor.matmul(fps[:], h_lo[:, j, :], wslab_t[:, j, 0, :],
                                     start=False, stop=False)
                    nc.tensor.matmul(fps[:], h_hi[:, j, :], wslab_t[:, j, 1, :],
                                     start=False, stop=(j == NJ - 1))
                nc.vector.tensor_copy(fc1s[:], fps[:])

            # ---- AllReduce fc1 partials ----
            arin = dr.tile([B, 120], F32)
            arout = dr.tile([B, 120], F32, addr_space="Shared")
            nc.sync.dma_start(arin[:], fc1s[:])
            nc.gpsimd.collective_compute(
                "AllReduce", ALU.add,
                replica_groups=[list(range(N_CORES))],
                ins=[arin.opt()], outs=[arout.opt()])
            h1post = sb.tile([B, 120], F32)
            nc.sync.dma_start(h1post[:], arout[:])

            # ---- tail (replicated) ----
            h1 = sb.tile([120, B], F32)
            h2 = sb.tile([84, B], F32)
            h10 = sb.tile([B, 1], F32)
            s1 = sb.tile([20, 1], F32)
            fs_row = sb.tile([1, 5], F32)
            fsb = sb.tile([128, 5], F32)
            diff = sb.tile([128, 64, 5], F32)
            sq = sb.tile([128, 64, 5], F32)
            d2 = sb.tile([128, 64], F32)
            kxv = sb.tile([128, 64], F32)
            pr = sb.tile([128, 2, 64], F32)
            krw = sb.tile([128, 2], F32)
            ones_t = sb.tile([128, 1], F32)
            out_sb = sb.tile([1, 2], F32)
            nc.vector.memset(ones_t[:], 1.0)

            with tc.tile_pool(name="ps_4", bufs=1, space="PSUM") as ps4:
                tp = ps4.tile([120, B], F32)
                nc.tensor.transpose(tp[:], h1post[:], small["idt10"][:])
                nc.scalar.activation(h1[:], tp[:], AF.Relu,
                                     bias=small["fc1b"][:])

                p2 = ps4.tile([84, B], F32)
                nc.tensor.matmul(p2[:], small["w2fcT"][:], h1[:],
                                 start=True, stop=True)
                nc.scalar.activation(h2[:], p2[:], AF.Relu,
                                     bias=small["fc2b"][:])

                p3 = ps4.tile([B, 1], F32)
                nc.tensor.matmul(p3[:], h2[:], small["w3fcT"][:],
                                 start=True, stop=True)
                nc.scalar.activation(h10[:], p3[:], AF.Identity,
                                     bias=small["b3vec"][:])

                p4 = ps4.tile([20, 1], F32)
                nc.tensor.matmul(p4[:], small["wq1T"][:], h10[:],
                                 start=True, stop=True)
                nc.scalar.activation(s1[:], p4[:], AF.Tanh)

                p5 = ps4.tile([1, 5], F32)
                nc.tensor.matmul(p5[:], s1[:], small["wq2T"][:],
                                 start=True, stop=True)
                nc.scalar.activation(fs_row[:], p5[:], AF.Tanh)

                nc.gpsimd.partition_broadcast(fsb[:], fs_row[0:1, :])
                nc.vector.tensor_sub(
                    diff[:], small["ts_r"][:],
                    fsb[:].unsqueeze(1).broadcast_to([128, 64, 5]))
                nc.vector.tensor_mul(sq[:], diff[:], diff[:])
                nc.vector.reduce_sum(d2[:], sq[:], axis=AX.X)
                nc.scalar.activation(kxv[:], d2[:], AF.Exp, scale=-1.0)
                nc.vector.tensor_mul(
                    pr[:], small["kcls_r"][:],
                    kxv[:].unsqueeze(1).broadcast_to([128, 2, 64]))
                nc.vector.reduce_sum(krw[:], pr[:], axis=AX.X)

                p6 = ps4.tile([1, 2], F32)
                nc.tensor.matmul(p6[:], ones_t[:], krw[:],
                                 start=True, stop=True)
                nc.vector.tensor_add(out_sb[:], p6[:], small["kclsb"][:])

            nc.sync.dma_start(out_d[:], out_sb[:])

    nc.compile()
    return nc


def _prep_inputs(inputs):
    f32 = np.float32
    x = np.asarray(inputs["x"], f32)
    conv1_w = np.asarray(inputs["conv1_w"], f32)
    conv1_b = np.asarray(inputs["conv1_b"], f32)
    conv2_w = np.asarray(inputs["conv2_w"], f32)
    conv2_b = np.asarray(inputs["conv2_b"], f32)
    fc1_w = np.asarray(inputs["fc1_w"], f32)
    fc1_b = np.asarray(inputs["fc1_b"], f32)
    fc2_w = np.asarray(inputs["fc2_w"], f32)
    fc2_b = np.asarray(inputs["fc2_b"], f32)
    fc3_w = np.asarray(inputs["fc3_w"], f32)
    fc3_b = np.asarray(inputs["fc3_b"], f32)
    qnn_w1 = np.asarray(inputs["qnn_w1"], f32)
    qnn_w2 = np.asarray(inputs["qnn_w2"], f32)
    ts = np.asarray(inputs["train_states"], f32)
    kcls_w = np.asarray(inputs["kcls_w"], f32)
    kcls_b = np.asarray(inputs["kcls_b"], f32)

    pack1 = np.zeros((128, 576), f32)
    pack1[0:120, 0:1] = fc1_b.reshape(120, 1)
    pack1[0:120, 1:85] = fc2_w.T
    pack1[0:84, 85:86] = fc2_b.reshape(84, 1)
    pack1[0:84, 86:87] = fc3_w.T
    pack1[0:B, 87:88] = fc3_b[0]
    pack1[0:B, 88:108] = qnn_w1.T
    pack1[0:20, 108:113] = qnn_w2.T
    pack1[0:B, 113:123] = np.eye(B, dtype=f32)
    pack1[0:1, 123:125] = kcls_b.reshape(1, 2)
    pack1[:, 128:448] = ts.reshape(128, 320)
    pack1[:, 448:576] = kcls_w.reshape(2, 128, 64).transpose(1, 0, 2).reshape(128, 128)
    shared = {"pack1": pack1}

    fc1_w4 = fc1_w.reshape(120, 15, 61, 61)

    in_maps = []
    for a, b in BANDS:
        nb = b - a
        Y0 = 2 * a - 1          # conv1 row of y_loc 0 (also pool1 row of py_loc 0)
        X0 = 4 * a - 3          # x row of r_loc 0

        # x slabs: x2 = [c0 rows | c1 rows | ones], x3 = [c2 rows]
        xs = np.zeros((3, XR, B, XC), f32)
        r_lo = max(0, X0)
        r_hi = min(250, X0 + XR)
        xs[:, r_lo - X0: r_hi - X0, :, 1:251] = (
            x[:, :, r_lo:r_hi, :].transpose(1, 2, 0, 3))
        x2 = np.concatenate(
            [xs[0], xs[1], np.ones((1, B, XC), f32)], axis=0)
        x3 = xs[2]

        # conv1 banded weights: K=(c, r_loc)+bias, M=(y_loc, och), per kx
        w1 = np.zeros((3, 43, 5, 120), f32)     # [c, r_loc, kx, m=(y_loc,och)]
        for y_loc in range(C1R):
            y = Y0 + y_loc
            if not (0 <= y <= 123):
                continue
            for ky in range(5):
                r_loc = 2 * y_loc + ky
                if r_loc >= XR:
                    continue
                for c in range(3):
                    w1[c, r_loc, :, y_loc * 6: y_loc * 6 + 6] = \
                        conv1_w[:, c, ky, :].T
        w1a = np.zeros((87, 5, 120), f32)
        w1a[0:43] = w1[0]
        w1a[43:86] = w1[1]
        w1a[86, 0, :] = np.tile(conv1_b, C1R)   # bias row, kx=0 only
        w1b = np.ascontiguousarray(w1[2])

        # conv2 banded weights: K=(py_loc, ich)+bias@114, M=(i2_loc, och2)
        w2 = np.zeros((115, 3, 135), f32)
        for i2_loc in range(C2R):
            i2 = a + i2_loc
            if i2 > 61:
                continue
            for kyp in range(3):
                py_loc = 2 * i2_loc + kyp
                py = Y0 + py_loc
                if py_loc >= P1R or not (0 <= py <= 122):
                    continue
                for ich in range(6):
                    q = py_loc * 6 + ich
                    m0 = i2_loc * 15
                    w2[q, :, m0:m0 + 15] = conv2_w[:, ich, kyp, :].T
        w2[114, 0, :] = np.tile(conv2_b, 9)     # bias row, kxp=0 only

        # partition-shift matrices
        s1m = np.zeros((120, 114), f32)
        for m in range(114):
            s1m[m + 6, m] = 1.0
        s2a = np.zeros((120, 120), f32)
        s2b = np.zeros((15, 120), f32)
        for m in range(105):
            s2a[m + 15, m] = 1.0
        for m in range(105, 120):
            s2b[m - 105, m] = 1.0

        # fc1 weight slab, split bf16: [p=(i2_loc,och2), j, {hi,lo}, och1]
        wsl = np.zeros((8, 15, NJ, 120), f32)
        nrow = min(nb, 8)
        wsl[0:nrow] = fc1_w4[:, :, a:a + nrow, :].transpose(2, 1, 3, 0)
        wsl = wsl.reshape(120, NJ, 120)
        hi = wsl.astype(ml_dtypes.bfloat16)
        lo = (wsl - hi.astype(f32)).astype(ml_dtypes.bfloat16)
        wslab = np.stack([hi, lo], axis=2)      # [120, NJ, 2, 120] bf16

        pack2 = np.zeros((120, 768), f32)
        pack2[0:120, 0:114] = s1m
        pack2[0:120, 114:234] = s2a
        pack2[0:15, 234:354] = s2b
        pack2[0:115, 354:759] = w2.reshape(115, 405)

        m = dict(shared)
        m["ones_v"] = np.ones((1, B, 125), f32)
        m["zpad_v"] = np.zeros((114, B, 2), f32)
        m.update({"x2": x2, "x3": x3, "w1a": w1a, "w1b": w1b,
                  "pack2": pack2, "wslab": np.ascontiguousarray(wslab)})
        in_maps.append(m)
    return in_maps


_NC_CACHE = None


def kernel(**inputs) -> np.ndarray:
    global _NC_CACHE
    if _NC_CACHE is None:
        _NC_CACHE = _build_nc()
    nc = _NC_CACHE
    in_maps = _prep_inputs(inputs)
    res = bass_utils.run_bass_kernel_spmd(
        nc, in_maps, core_ids=list(range(N_CORES)))
    return res.results[0]["out"]
